# revision 14
# baseline (speedup 1.0000x reference)
"""ComplexMixture Trainium2 kernel.

Computes, for each batch b of input_real/input_imag [B, S, D]:
    out_real[b] = (R^T R + I^T I) / S          (symmetric   [D, D])
    out_imag[b] = (R^T I - (R^T I)^T) / S      (antisym     [D, D])
with B=32, S=8192, D=64.

Strategy: data-parallel over batch across 8 NeuronCores (4 batches/core).
Host packs Z = [R | I] ([S, 2D]) per batch; all per-batch outputs derive
from the Gram matrix G = Z^T Z ([128, 128]) = [[rr, ri], [ri^T, ii]].

Given (scaled) G in SBUF, a tiny "shift" matmul H = J64^T G (J64 = rows
64:128 of the 128-identity) moves the bottom 64 partitions of G up so the
block combines are elementwise:
    out_real = G[0:64, 0:64] + H[:, 64:128]
    out_imag = G[0:64, 64:128] - H[:, 0:64]

Variants (VARIANT):
  "fp16" (fastest, ~2e-4 rel err): single fp16 Gram; 2 bytes/element of
    DMA; one 1-cycle/row matmul per k-tile.
  "fp16f8" (default; ~1e-5, ~25% slower): Z = Zh + Zl/LS8 with Zh =
    fp16(Z) and Zl = fp8e4m3((Z - Zh) * LS8).  The fp8 lo part is cast
    to fp16 during its (SWDGE) DMA.  Using C = Zh^T Zl and hl+lh = C+C^T,
        G = Zh^T Zh + (C + C^T)/LS8 + O(2^-15)
    so one N=256 matmul per k-tile (rhs = [Zh|Zl], weights loaded once)
    plus one PE transpose per batch. 3 bytes/element of DMA.
  "fp16hl" (~1e-6): same but lo part is fp16 (scaled 2^11); 4 B/elem.
  "fp32" (exact, slowest): plain fp32 Gram (4 cycles/row, 4 B/elem).

Inputs stream in ~1-2 MiB fully-contiguous chunks issued on the Sync
HWDGE ring only (FIFO -> in-order completion, so the PE starts after the
first chunk); the last batch ends with a small chunk to shrink the
end-of-kernel lag.  Consts ride the Scalar ring; outputs accumulate in
one SBUF tile and leave in a single DMA (host re-transposes).
"""

import os
import numpy as np
import ml_dtypes

import concourse.bass as bass
import concourse.tile as tile
from concourse import bacc, mybir
from concourse.bass_utils import run_bass_kernel_spmd

B, S, D = 32, 8192, 64
D2 = 2 * D                  # packed feature width (R|I)
N_CORES = 8
BPC = B // N_CORES          # batches per core
P = 128                     # partitions / K-tile size
T = S // P                  # K-tiles per batch
INV_S = 1.0 / S
LSCALE = 2048.0             # lo-part scale (2^11)

VARIANT = os.environ.get("KERNEL_VARIANT", "fp16f8")

# Per-batch chunk patterns (k-tiles per chunk).  2-streams-per-elem
# variants use 16-tile chunks (~2.1 MB), 1-stream use 32-tile (~2.1 MB
# fp32 / ~1.05 MB fp16).  Last batch tapers so the final chunk is small.
CHUNKS_2 = [[16, 16, 16, 16]] * (BPC - 1) + [[16, 16, 16, 12, 4]]
CHUNKS_1 = [[32, 32]] * (BPC - 1) + [[32, 24, 8]]

_NC_CACHE = {}
LAST_RESULTS = None         # BassKernelResults of the most recent run


def _shift_combine(nc, gpool, psh, j64_sb, g_sb, o_all, b):
    """Given scaled G in SBUF ([128,128] f32), write batch b of o_all."""
    h_ps = psh.tile([D, P], mybir.dt.float32)
    nc.tensor.matmul(h_ps[:], j64_sb[:], g_sb[:], start=True, stop=True)

    nc.vector.tensor_add(o_all[:, b, 0, :], g_sb[0:D, 0:D], h_ps[:, D : 2 * D])
    nc.vector.tensor_sub(o_all[:, b, 1, :], g_sb[0:D, D : 2 * D], h_ps[:, 0:D])


def _chunk_sizes(pattern, width):
    return [nt * P * width for nt in pattern]


def _build_nc_hl(lo_fp8):
    """fp16 hi/lo 2-matmul variant; lo arrives as fp8 (cast in DMA) or fp16."""
    nc = bacc.Bacc("TRN2", target_bir_lowering=False, debug=False)

    if lo_fp8:
        xh = nc.dram_tensor(
            "xh", [BPC * S * D2], mybir.dt.float16, kind="ExternalInput"
        )
        xl = nc.dram_tensor(
            "xl", [BPC * S * D2], mybir.dt.float8e4, kind="ExternalInput"
        )
    else:
        xh = nc.dram_tensor(
            "xh", [BPC * S * 2 * D2], mybir.dt.float16, kind="ExternalInput"
        )
        xl = None
    j64 = nc.dram_tensor("j64", [P, D], mybir.dt.float32, kind="ExternalInput")
    id128 = nc.dram_tensor("id128", [P, P], mybir.dt.float32, kind="ExternalInput")
    out = nc.dram_tensor("out", [D, BPC, 2, D], mybir.dt.float32, kind="ExternalOutput")

    with tile.TileContext(nc) as tc:
        with (
            tc.tile_pool(name="consts", bufs=1) as consts,
            tc.tile_pool(name="zpool", bufs=10) as zpool,
            tc.tile_pool(name="gpool", bufs=4) as gpool,
            tc.tile_pool(name="opool", bufs=1) as opool,
            tc.tile_pool(name="psg", bufs=2, space="PSUM") as psg,
            tc.tile_pool(name="psct", bufs=2, space="PSUM") as psct,
            tc.tile_pool(name="psh", bufs=2, space="PSUM") as psh,
        ):
            j64_sb = consts.tile([P, D], mybir.dt.float32)
            nc.scalar.dma_start(out=j64_sb[:], in_=j64[:])
            id_sb = consts.tile([P, P], mybir.dt.float32)
            nc.scalar.dma_start(out=id_sb[:], in_=id128[:])
            o_all = opool.tile([D, BPC, 2, D], mybir.dt.float32)

            off = 0
            for b in range(BPC):
                zc = []
                for ci, nt in enumerate(CHUNKS_2[b]):
                    z = zpool.tile(
                        [P, nt, 2, D2], mybir.dt.float16,
                        name=f"z_{b}_{ci}", tag="z",
                    )
                    n = nt * P * D2
                    if lo_fp8:
                        nc.sync.dma_start(
                            out=z[:, :, 0, :],
                            in_=xh[off : off + n].rearrange(
                                "(p t c) -> p t c", p=P, t=nt
                            ),
                        )
                        nc.gpsimd.dma_start(   # SWDGE: fp8 -> fp16 cast in DMA
                            out=z[:, :, 1, :],
                            in_=xl[off : off + n].rearrange(
                                "(p t c) -> p t c", p=P, t=nt
                            ),
                        )
                        off += n
                    else:
                        nc.sync.dma_start(
                            out=z[:],
                            in_=xh[2 * off : 2 * off + 2 * n].rearrange(
                                "(p t h c) -> p t h c", p=P, t=nt, h=2
                            ),
                        )
                        off += n
                    zc.append((z, nt))

                # g1 = Zh^T [Zh | Zl]:  A = g1[:, :128] = hh, C = g1[:, 128:] = hl
                g1_ps = psg.tile([P, 2 * P], mybir.dt.float32)
                first = True
                nchunks = len(zc)
                for ci, (z, nt) in enumerate(zc):
                    for t in range(nt):
                        nc.tensor.matmul(
                            g1_ps[:],
                            z[:, t, 0, :],       # lhsT = Zh_t [128, 128]
                            z[:, t, :, :],       # rhs  = [Zh_t | Zl_t] [128, 256]
                            start=first,
                            stop=(ci == nchunks - 1 and t == nt - 1),
                        )
                        first = False

                # cs = C * (inv_s / LSCALE)
                cs = gpool.tile([P, P], mybir.dt.float32, name=f"cs_{b}", tag="cs")
                nc.vector.tensor_scalar_mul(cs[:], g1_ps[:, P : 2 * P], INV_S / LSCALE)
                # ct = cs^T (PE transpose; already scaled)
                ct_ps = psct.tile([P, P], mybir.dt.float32)
                nc.tensor.transpose(ct_ps[:], cs[:], id_sb[:])
                # g2 = A*inv_s + cs + ct   (scaled G)
                g_sb = gpool.tile([P, P], mybir.dt.float32, name=f"g_sb_{b}", tag="g")
                nc.vector.scalar_tensor_tensor(
                    out=g_sb[:],
                    in0=g1_ps[:, 0:P],
                    scalar=INV_S,
                    in1=cs[:],
                    op0=mybir.AluOpType.mult,
                    op1=mybir.AluOpType.add,
                )
                g2_sb = gpool.tile([P, P], mybir.dt.float32, name=f"g2_{b}", tag="g2")
                nc.vector.tensor_add(g2_sb[:], g_sb[:], ct_ps[:])

                _shift_combine(nc, gpool, psh, j64_sb, g2_sb, o_all, b)

            nc.scalar.dma_start(out=out[:], in_=o_all[:])

    nc.compile()
    return nc


def _build_nc_1s(dt_in):
    """Single-stream Gram (fp16 or fp32 k-tiles), one MM per k-tile."""
    nc = bacc.Bacc("TRN2", target_bir_lowering=False, debug=False)

    xh = nc.dram_tensor("xh", [BPC * S * D2], dt_in, kind="ExternalInput")
    j64 = nc.dram_tensor("j64", [P, D], mybir.dt.float32, kind="ExternalInput")
    out = nc.dram_tensor("out", [D, BPC, 2, D], mybir.dt.float32, kind="ExternalOutput")

    with tile.TileContext(nc) as tc:
        with (
            tc.tile_pool(name="consts", bufs=1) as consts,
            tc.tile_pool(name="zpool", bufs=6) as zpool,
            tc.tile_pool(name="gpool", bufs=2) as gpool,
            tc.tile_pool(name="opool", bufs=1) as opool,
            tc.tile_pool(name="psg", bufs=2, space="PSUM") as psg,
            tc.tile_pool(name="psh", bufs=2, space="PSUM") as psh,
        ):
            j64_sb = consts.tile([P, D], mybir.dt.float32)
            nc.scalar.dma_start(out=j64_sb[:], in_=j64[:])
            o_all = opool.tile([D, BPC, 2, D], mybir.dt.float32)

            off = 0
            for b in range(BPC):
                zc = []
                for ci, nt in enumerate(CHUNKS_1[b]):
                    z = zpool.tile(
                        [P, nt, D2], dt_in, name=f"z_{b}_{ci}", tag="z"
                    )
                    n = nt * P * D2
                    nc.sync.dma_start(
                        out=z[:],
                        in_=xh[off : off + n].rearrange(
                            "(p t c) -> p t c", p=P, t=nt
                        ),
                    )
                    off += n
                    zc.append((z, nt))

                g_ps = psg.tile([P, P], mybir.dt.float32)
                first = True
                nchunks = len(zc)
                for ci, (z, nt) in enumerate(zc):
                    for t in range(nt):
                        zt = z[:, t, :]
                        nc.tensor.matmul(
                            g_ps[:], zt, zt,
                            start=first,
                            stop=(ci == nchunks - 1 and t == nt - 1),
                        )
                        first = False

                g_sb = gpool.tile([P, P], mybir.dt.float32, name=f"g_sb_{b}", tag="g")
                nc.vector.tensor_scalar_mul(g_sb[:], g_ps[:], INV_S)
                _shift_combine(nc, gpool, psh, j64_sb, g_sb, o_all, b)

            nc.scalar.dma_start(out=out[:], in_=o_all[:])

    nc.compile()
    return nc


def _flat_chunks(patterns):
    """Yield (b, ci, nt, off, first_of_batch, last_of_batch) over batches."""
    off = 0
    for b in range(BPC):
        n = len(patterns[b])
        for ci, nt in enumerate(patterns[b]):
            yield b, ci, nt, off, ci == 0, ci == n - 1
            off += nt * P * D2


def _build_nc_fp16_raw():
    """Hand-synchronized raw-bass fp16 Gram: no Tile boot/teardown cost.

    Sync engine: 9 chunk DMAs (unique SBUF slot each, FIFO ring).
    Tensor: per batch 64 accumulating MMs (+ J-shift MM, scheduled after
    the next batch's first chunk to hide the DVE round-trip).
    Vector: per batch scale-copy of G then the two block combines.
    Scalar: consts in, one packed output DMA out.
    """
    from contextlib import ExitStack

    nc = bacc.Bacc("TRN2", target_bir_lowering=False, debug=False)

    xh = nc.dram_tensor("xh", [BPC * S * D2], mybir.dt.float16, kind="ExternalInput")
    j64 = nc.dram_tensor("j64", [P, D], mybir.dt.float32, kind="ExternalInput")
    out = nc.dram_tensor("out", [D, BPC, 2, D], mybir.dt.float32, kind="ExternalOutput")

    chunks = list(_flat_chunks(CHUNKS_1))
    NCH = len(chunks)

    with ExitStack() as es:
        e = es.enter_context
        z = [
            e(nc.sbuf_tensor(f"z{k}", [P, nt, D2], mybir.dt.float16))
            for k, (_, _, nt, _, _, _) in enumerate(chunks)
        ]
        g_ps = [e(nc.psum_tensor(f"gps{i}", [P, P], mybir.dt.float32)) for i in range(2)]
        h_ps = [e(nc.psum_tensor(f"hps{i}", [D, P], mybir.dt.float32)) for i in range(2)]
        g_sb = [e(nc.sbuf_tensor(f"gsb{i}", [P, P], mybir.dt.float32)) for i in range(2)]
        o_all = e(nc.sbuf_tensor("o_all", [D, BPC, 2, D], mybir.dt.float32))
        j64_sb = e(nc.sbuf_tensor("j64sb", [P, D], mybir.dt.float32))

        dsem = [e(nc.semaphore(f"d{k}")) for k in range(NCH)]
        csem = e(nc.semaphore("csem"))
        pe_g = e(nc.semaphore("pe_g"))
        vec_g = e(nc.semaphore("vec_g"))
        pe_h = e(nc.semaphore("pe_h"))
        vec_o = e(nc.semaphore("vec_o"))
        osem = e(nc.semaphore("osem"))

        with nc.Block() as block:

            @block.sync
            def _(sync):
                for k, (_, _, nt, off, _, _) in enumerate(chunks):
                    if k % 2 != 0:
                        continue
                    n = nt * P * D2
                    sync.dma_start(
                        out=z[k][:],
                        in_=xh[off : off + n].rearrange(
                            "(p t c) -> p t c", p=P, t=nt
                        ),
                    ).then_inc(dsem[k], 16)

            @block.scalar
            def _(scalar):
                scalar.dma_start(out=j64_sb[:], in_=j64[:]).then_inc(csem, 16)
                for k, (_, _, nt, off, _, _) in enumerate(chunks):
                    if k % 2 != 1:
                        continue
                    n = nt * P * D2
                    scalar.dma_start(
                        out=z[k][:],
                        in_=xh[off : off + n].rearrange(
                            "(p t c) -> p t c", p=P, t=nt
                        ),
                    ).then_inc(dsem[k], 16)
                scalar.wait_ge(vec_o, BPC)
                scalar.dma_start(out=out[:], in_=o_all[:]).then_inc(osem, 16)
                scalar.wait_ge(osem, 16)

            @block.tensor
            def _(tensor):
                def jmm(b):
                    # h = J64^T G_b ; h_ps[b%2] free once batch b-2 combined
                    tensor.wait_ge(vec_g, b + 1)
                    if b >= 1:
                        tensor.wait_ge(vec_o, b)
                    if b == 0:
                        tensor.wait_ge(csem, 16)
                    tensor.matmul(
                        h_ps[b % 2][:], j64_sb[:], g_sb[b % 2][:],
                        start=True, stop=True, skip_group_check=True,
                    ).then_inc(pe_h, 1)

                for k, (b, ci, nt, off, first_c, last_c) in enumerate(chunks):
                    if first_c and b >= 2:
                        tensor.wait_ge(vec_g, b - 1)  # g_ps[b%2] drained
                    tensor.wait_ge(dsem[k], 16)
                    for t in range(nt):
                        zt = z[k][:, t, :]
                        mm = tensor.matmul(
                            g_ps[b % 2][:], zt, zt,
                            start=(first_c and t == 0),
                            stop=(last_c and t == nt - 1),
                            skip_group_check=True,
                        )
                        if last_c and t == nt - 1:
                            mm.then_inc(pe_g, 1)
                    if first_c and b >= 1:
                        jmm(b - 1)  # hide DVE round-trip behind this chunk
                jmm(BPC - 1)

            @block.vector
            def _(vector):
                for b in range(BPC):
                    vector.wait_ge(pe_g, b + 1)
                    nc.vector.tensor_scalar_mul(
                        g_sb[b % 2][:], g_ps[b % 2][:], INV_S
                    ).then_inc(vec_g, 1)
                    vector.wait_ge(pe_h, b + 1)
                    nc.vector.tensor_add(
                        o_all[:, b, 0, :],
                        g_sb[b % 2][0:D, 0:D],
                        h_ps[b % 2][:, D : 2 * D],
                    )
                    nc.vector.tensor_sub(
                        o_all[:, b, 1, :],
                        g_sb[b % 2][0:D, D : 2 * D],
                        h_ps[b % 2][:, 0:D],
                    ).then_inc(vec_o, 1)

    nc.compile()
    return nc


def _build_nc_hl_raw():
    """Raw-bass fp16 hi/lo 2-matmul variant (fp32-grade accuracy)."""
    from contextlib import ExitStack

    nc = bacc.Bacc("TRN2", target_bir_lowering=False, debug=False)

    xh = nc.dram_tensor(
        "xh", [BPC * S * 2 * D2], mybir.dt.float16, kind="ExternalInput"
    )
    j64 = nc.dram_tensor("j64", [P, D], mybir.dt.float32, kind="ExternalInput")
    id128 = nc.dram_tensor("id128", [P, P], mybir.dt.float32, kind="ExternalInput")
    out = nc.dram_tensor("out", [D, BPC, 2, D], mybir.dt.float32, kind="ExternalOutput")

    chunks = list(_flat_chunks(CHUNKS_2))
    NCH = len(chunks)
    NSLOT = 8
    MAXT = max(nt for (_, _, nt, _, _, _) in chunks)

    with ExitStack() as es:
        e = es.enter_context
        z = [
            e(nc.sbuf_tensor(f"z{i}", [P, MAXT, 2, D2], mybir.dt.float16))
            for i in range(NSLOT)
        ]
        g1_ps = [e(nc.psum_tensor(f"g1ps{i}", [P, 2 * P], mybir.dt.float32)) for i in range(2)]
        ct_ps = [e(nc.psum_tensor(f"ctps{i}", [P, P], mybir.dt.float32)) for i in range(2)]
        h_ps = [e(nc.psum_tensor(f"hps{i}", [D, P], mybir.dt.float32)) for i in range(2)]
        cs_sb = [e(nc.sbuf_tensor(f"cssb{i}", [P, P], mybir.dt.float32)) for i in range(2)]
        g2_sb = [e(nc.sbuf_tensor(f"g2sb{i}", [P, P], mybir.dt.float32)) for i in range(2)]
        o_all = e(nc.sbuf_tensor("o_all", [D, BPC, 2, D], mybir.dt.float32))
        j64_sb = e(nc.sbuf_tensor("j64sb", [P, D], mybir.dt.float32))
        id_sb = e(nc.sbuf_tensor("idsb", [P, P], mybir.dt.float32))

        dsem = [e(nc.semaphore(f"d{k}")) for k in range(NCH)]
        cons = e(nc.semaphore("cons"))
        csem = e(nc.semaphore("csem"))
        vec_cs = e(nc.semaphore("vec_cs"))
        pe_ct = e(nc.semaphore("pe_ct"))
        vec_g2 = e(nc.semaphore("vec_g2"))
        pe_h = e(nc.semaphore("pe_h"))
        vec_o = e(nc.semaphore("vec_o"))
        osem = e(nc.semaphore("osem"))

        with nc.Block() as block:

            @block.sync
            def _(sync):
                for k, (_, _, nt, off, _, _) in enumerate(chunks):
                    if k >= NSLOT:
                        sync.wait_ge(cons, k - NSLOT + 1)
                    n = nt * P * 2 * D2
                    sync.dma_start(
                        out=z[k % NSLOT][:, :nt, :, :],
                        in_=xh[2 * off : 2 * off + n].rearrange(
                            "(p t h c) -> p t h c", p=P, t=nt, h=2
                        ),
                    ).then_inc(dsem[k], 16)

            @block.scalar
            def _(scalar):
                scalar.dma_start(out=j64_sb[:], in_=j64[:]).then_inc(csem, 16)
                scalar.dma_start(out=id_sb[:], in_=id128[:]).then_inc(csem, 16)
                scalar.wait_ge(vec_o, BPC)
                scalar.dma_start(out=out[:], in_=o_all[:]).then_inc(osem, 16)
                scalar.wait_ge(osem, 16)

            @block.tensor
            def _(tensor):
                def ctmm(b):
                    # ct = cs^T (needs id128)
                    tensor.wait_ge(vec_cs, b + 1)
                    if b == 0:
                        tensor.wait_ge(csem, 32)
                    if b >= 2:
                        tensor.wait_ge(vec_g2, b - 1)  # ct_ps[b%2] drained
                    tensor.transpose(
                        ct_ps[b % 2][:], cs_sb[b % 2][:], id_sb[:]
                    ).then_inc(pe_ct, 1)

                def jmm(b):
                    tensor.wait_ge(vec_g2, b + 1)
                    if b >= 1:
                        tensor.wait_ge(vec_o, b)
                    tensor.matmul(
                        h_ps[b % 2][:], j64_sb[:], g2_sb[b % 2][:],
                        start=True, stop=True, skip_group_check=True,
                    ).then_inc(pe_h, 1)

                for k, (b, ci, nt, off, first_c, last_c) in enumerate(chunks):
                    if first_c and b >= 2:
                        tensor.wait_ge(vec_cs, b - 1)  # g1_ps[b%2] cs read
                        tensor.wait_ge(vec_g2, b - 1)  # g1_ps[b%2] A read
                    tensor.wait_ge(dsem[k], 16)
                    for t in range(nt):
                        mm = tensor.matmul(
                            g1_ps[b % 2][:],
                            z[k % NSLOT][:, t, 0, :],
                            z[k % NSLOT][:, t, :, :],
                            start=(first_c and t == 0),
                            stop=(last_c and t == nt - 1),
                            skip_group_check=True,
                        )
                        if t == nt - 1:
                            mm.then_inc(cons, 1)
                    # hide DVE round-trips behind subsequent chunks
                    if b >= 1 and ci == 0:
                        ctmm(b - 1)
                    if b >= 1 and ci == 1:
                        jmm(b - 1)
                ctmm(BPC - 1)
                jmm(BPC - 1)

            @block.vector
            def _(vector):
                cum = 0
                for b in range(BPC):
                    cum += len(CHUNKS_2[b])
                    vector.wait_ge(cons, cum)
                    nc.vector.tensor_scalar_mul(
                        cs_sb[b % 2][:], g1_ps[b % 2][:, P : 2 * P], INV_S / LSCALE
                    ).then_inc(vec_cs, 1)
                    vector.wait_ge(pe_ct, b + 1)
                    if b >= 2:
                        vector.wait_ge(pe_h, b - 1)  # g2_sb[b%2] consumed
                    nc.vector.scalar_tensor_tensor(
                        out=g2_sb[b % 2][:],
                        in0=g1_ps[b % 2][:, 0:P],
                        scalar=INV_S,
                        in1=cs_sb[b % 2][:],
                        op0=mybir.AluOpType.mult,
                        op1=mybir.AluOpType.add,
                    )
                    nc.vector.tensor_add(
                        g2_sb[b % 2][:], g2_sb[b % 2][:], ct_ps[b % 2][:]
                    ).then_inc(vec_g2, 1)
                    vector.wait_ge(pe_h, b + 1)
                    nc.vector.tensor_add(
                        o_all[:, b, 0, :],
                        g2_sb[b % 2][0:D, 0:D],
                        h_ps[b % 2][:, D : 2 * D],
                    )
                    nc.vector.tensor_sub(
                        o_all[:, b, 1, :],
                        g2_sb[b % 2][0:D, D : 2 * D],
                        h_ps[b % 2][:, 0:D],
                    ).then_inc(vec_o, 1)

    nc.compile()
    return nc


def _j64_host():
    j = np.zeros((P, D), np.float32)
    j[D + np.arange(D), np.arange(D)] = 1.0
    return j


def _chunkify(a, patterns):
    """a: [BPC, S, ...tail] -> flat 1-D array in chunk layout.

    Chunk of nt k-tiles covering rows [base, base+nt*P): stored as
    [p, t, ...tail] with row = base + p*nt + t.
    """
    segs = []
    for b in range(BPC):
        base = 0
        for nt in patterns[b]:
            rows = nt * P
            seg = a[b, base : base + rows]          # [rows, ...tail]
            seg = seg.reshape(P, nt, *a.shape[2:])  # p-major
            segs.append(seg.reshape(-1))
            base += rows
    return np.concatenate(segs)


def _prep(xz):
    """Returns dict of per-core host arrays for the active VARIANT."""
    xzc = xz.reshape(N_CORES, BPC, S, D2)
    maps = []
    for c in range(N_CORES):
        a = xzc[c]
        if VARIANT in ("fp16", "fp16_raw"):
            m = {"xh": _chunkify(a.astype(np.float16), CHUNKS_1)}
        elif VARIANT == "fp32":
            m = {"xh": _chunkify(a, CHUNKS_1)}
        elif VARIANT == "fp16f8":
            zh = a.astype(np.float16)
            zl = ((a - zh.astype(np.float32)) * LSCALE).astype(
                ml_dtypes.float8_e4m3
            )
            m = {
                "xh": _chunkify(zh, CHUNKS_2),
                "xl": _chunkify(zl, CHUNKS_2),
            }
        elif VARIANT in ("fp16hl", "fp16hl_raw"):
            zh = a.astype(np.float16)
            zl = ((a - zh.astype(np.float32)) * LSCALE).astype(np.float16)
            zs = np.stack([zh, zl], axis=2)  # [BPC, S, 2, D2]
            m = {"xh": _chunkify(zs, CHUNKS_2)}
        else:
            raise ValueError(VARIANT)
        maps.append(m)
    return maps


def _build():
    if VARIANT == "fp16":
        return _build_nc_1s(mybir.dt.float16)
    if VARIANT == "fp16_raw":
        return _build_nc_fp16_raw()
    if VARIANT == "fp16hl_raw":
        return _build_nc_hl_raw()
    if VARIANT == "fp32":
        return _build_nc_1s(mybir.dt.float32)
    if VARIANT == "fp16f8":
        return _build_nc_hl(lo_fp8=True)
    if VARIANT == "fp16hl":
        return _build_nc_hl(lo_fp8=False)
    raise ValueError(VARIANT)


def kernel(input_real, input_imag):
    global LAST_RESULTS
    xr = np.asarray(input_real, dtype=np.float32)
    xi = np.asarray(input_imag, dtype=np.float32)
    assert xr.shape == (B, S, D) and xi.shape == (B, S, D)

    xz = np.concatenate([xr, xi], axis=2)  # [B, S, 2D]

    key = ("nc", VARIANT)
    if key not in _NC_CACHE:
        _NC_CACHE[key] = _build()
    nc = _NC_CACHE[key]

    maps = _prep(xz)
    j64 = _j64_host()
    ident = np.eye(P, dtype=np.float32)
    in_maps = []
    for c in range(N_CORES):
        m = dict(maps[c])
        m["j64"] = j64
        if VARIANT in ("fp16f8", "fp16hl", "fp16hl_raw"):
            m["id128"] = ident
        in_maps.append(m)
    tmpdir = os.environ.get("BASS_TMPDIR") or None
    res = run_bass_kernel_spmd(
        nc, in_maps, core_ids=list(range(N_CORES)), tmpdir=tmpdir
    )
    LAST_RESULTS = res

    # per-core out: [D, BPC, 2, D] -> [BPC, 2, D, D]
    outs = np.stack(
        [res.results[c]["out"].transpose(1, 2, 0, 3) for c in range(N_CORES)]
    )
    out = outs.reshape(B, 2, D, D)
    return np.ascontiguousarray(out[:, 0]), np.ascontiguousarray(out[:, 1])


# revision 15
# speedup vs baseline: 1.0311x; 1.0311x over previous
"""ComplexMixture Trainium2 kernel.

Computes, for each batch b of input_real/input_imag [B, S, D]:
    out_real[b] = (R^T R + I^T I) / S          (symmetric   [D, D])
    out_imag[b] = (R^T I - (R^T I)^T) / S      (antisym     [D, D])
with B=32, S=8192, D=64.

Strategy: data-parallel over batch across 8 NeuronCores (4 batches/core).
Host packs Z = [R | I] ([S, 2D]) per batch; all per-batch outputs derive
from the Gram matrix G = Z^T Z ([128, 128]) = [[rr, ri], [ri^T, ii]].

Given (scaled) G in SBUF, a tiny "shift" matmul H = J64^T G (J64 = rows
64:128 of the 128-identity) moves the bottom 64 partitions of G up so the
block combines are elementwise:
    out_real = G[0:64, 0:64] + H[:, 64:128]
    out_imag = G[0:64, 64:128] - H[:, 0:64]

Variants (VARIANT):
  "fp16" (fastest, ~2e-4 rel err): single fp16 Gram; 2 bytes/element of
    DMA; one 1-cycle/row matmul per k-tile.
  "fp16f8" (default; ~1e-5, ~25% slower): Z = Zh + Zl/LS8 with Zh =
    fp16(Z) and Zl = fp8e4m3((Z - Zh) * LS8).  The fp8 lo part is cast
    to fp16 during its (SWDGE) DMA.  Using C = Zh^T Zl and hl+lh = C+C^T,
        G = Zh^T Zh + (C + C^T)/LS8 + O(2^-15)
    so one N=256 matmul per k-tile (rhs = [Zh|Zl], weights loaded once)
    plus one PE transpose per batch. 3 bytes/element of DMA.
  "fp16hl" (~1e-6): same but lo part is fp16 (scaled 2^11); 4 B/elem.
  "fp32" (exact, slowest): plain fp32 Gram (4 cycles/row, 4 B/elem).

Inputs stream in ~1-2 MiB fully-contiguous chunks issued on the Sync
HWDGE ring only (FIFO -> in-order completion, so the PE starts after the
first chunk); the last batch ends with a small chunk to shrink the
end-of-kernel lag.  Consts ride the Scalar ring; outputs accumulate in
one SBUF tile and leave in a single DMA (host re-transposes).
"""

import os
import numpy as np
import ml_dtypes

import concourse.bass as bass
import concourse.tile as tile
from concourse import bacc, mybir
from concourse.bass_utils import run_bass_kernel_spmd

B, S, D = 32, 8192, 64
D2 = 2 * D                  # packed feature width (R|I)
N_CORES = 8
BPC = B // N_CORES          # batches per core
P = 128                     # partitions / K-tile size
T = S // P                  # K-tiles per batch
INV_S = 1.0 / S
LSCALE = 2048.0             # lo-part scale (2^11)

VARIANT = os.environ.get("KERNEL_VARIANT", "fp16_raw")

# Per-batch chunk patterns (k-tiles per chunk).  2-streams-per-elem
# variants use 16-tile chunks (~2.1 MB), 1-stream use 32-tile (~2.1 MB
# fp32 / ~1.05 MB fp16).  Last batch tapers so the final chunk is small.
CHUNKS_2 = [[16, 16, 16, 16]] * (BPC - 1) + [[16, 16, 16, 12, 4]]
CHUNKS_1 = [[32, 32]] * (BPC - 1) + [[32, 24, 8]]

_NC_CACHE = {}
LAST_RESULTS = None         # BassKernelResults of the most recent run


def _shift_combine(nc, gpool, psh, j64_sb, g_sb, o_all, b):
    """Given scaled G in SBUF ([128,128] f32), write batch b of o_all."""
    h_ps = psh.tile([D, P], mybir.dt.float32)
    nc.tensor.matmul(h_ps[:], j64_sb[:], g_sb[:], start=True, stop=True)

    nc.vector.tensor_add(o_all[:, b, 0, :], g_sb[0:D, 0:D], h_ps[:, D : 2 * D])
    nc.vector.tensor_sub(o_all[:, b, 1, :], g_sb[0:D, D : 2 * D], h_ps[:, 0:D])


def _chunk_sizes(pattern, width):
    return [nt * P * width for nt in pattern]


def _build_nc_hl(lo_fp8):
    """fp16 hi/lo 2-matmul variant; lo arrives as fp8 (cast in DMA) or fp16."""
    nc = bacc.Bacc("TRN2", target_bir_lowering=False, debug=False)

    if lo_fp8:
        xh = nc.dram_tensor(
            "xh", [BPC * S * D2], mybir.dt.float16, kind="ExternalInput"
        )
        xl = nc.dram_tensor(
            "xl", [BPC * S * D2], mybir.dt.float8e4, kind="ExternalInput"
        )
    else:
        xh = nc.dram_tensor(
            "xh", [BPC * S * 2 * D2], mybir.dt.float16, kind="ExternalInput"
        )
        xl = None
    j64 = nc.dram_tensor("j64", [P, D], mybir.dt.float32, kind="ExternalInput")
    id128 = nc.dram_tensor("id128", [P, P], mybir.dt.float32, kind="ExternalInput")
    out = nc.dram_tensor("out", [D, BPC, 2, D], mybir.dt.float32, kind="ExternalOutput")

    with tile.TileContext(nc) as tc:
        with (
            tc.tile_pool(name="consts", bufs=1) as consts,
            tc.tile_pool(name="zpool", bufs=10) as zpool,
            tc.tile_pool(name="gpool", bufs=4) as gpool,
            tc.tile_pool(name="opool", bufs=1) as opool,
            tc.tile_pool(name="psg", bufs=2, space="PSUM") as psg,
            tc.tile_pool(name="psct", bufs=2, space="PSUM") as psct,
            tc.tile_pool(name="psh", bufs=2, space="PSUM") as psh,
        ):
            j64_sb = consts.tile([P, D], mybir.dt.float32)
            nc.scalar.dma_start(out=j64_sb[:], in_=j64[:])
            id_sb = consts.tile([P, P], mybir.dt.float32)
            nc.scalar.dma_start(out=id_sb[:], in_=id128[:])
            o_all = opool.tile([D, BPC, 2, D], mybir.dt.float32)

            off = 0
            for b in range(BPC):
                zc = []
                for ci, nt in enumerate(CHUNKS_2[b]):
                    z = zpool.tile(
                        [P, nt, 2, D2], mybir.dt.float16,
                        name=f"z_{b}_{ci}", tag="z",
                    )
                    n = nt * P * D2
                    if lo_fp8:
                        nc.sync.dma_start(
                            out=z[:, :, 0, :],
                            in_=xh[off : off + n].rearrange(
                                "(p t c) -> p t c", p=P, t=nt
                            ),
                        )
                        nc.gpsimd.dma_start(   # SWDGE: fp8 -> fp16 cast in DMA
                            out=z[:, :, 1, :],
                            in_=xl[off : off + n].rearrange(
                                "(p t c) -> p t c", p=P, t=nt
                            ),
                        )
                        off += n
                    else:
                        nc.sync.dma_start(
                            out=z[:],
                            in_=xh[2 * off : 2 * off + 2 * n].rearrange(
                                "(p t h c) -> p t h c", p=P, t=nt, h=2
                            ),
                        )
                        off += n
                    zc.append((z, nt))

                # g1 = Zh^T [Zh | Zl]:  A = g1[:, :128] = hh, C = g1[:, 128:] = hl
                g1_ps = psg.tile([P, 2 * P], mybir.dt.float32)
                first = True
                nchunks = len(zc)
                for ci, (z, nt) in enumerate(zc):
                    for t in range(nt):
                        nc.tensor.matmul(
                            g1_ps[:],
                            z[:, t, 0, :],       # lhsT = Zh_t [128, 128]
                            z[:, t, :, :],       # rhs  = [Zh_t | Zl_t] [128, 256]
                            start=first,
                            stop=(ci == nchunks - 1 and t == nt - 1),
                        )
                        first = False

                # cs = C * (inv_s / LSCALE)
                cs = gpool.tile([P, P], mybir.dt.float32, name=f"cs_{b}", tag="cs")
                nc.vector.tensor_scalar_mul(cs[:], g1_ps[:, P : 2 * P], INV_S / LSCALE)
                # ct = cs^T (PE transpose; already scaled)
                ct_ps = psct.tile([P, P], mybir.dt.float32)
                nc.tensor.transpose(ct_ps[:], cs[:], id_sb[:])
                # g2 = A*inv_s + cs + ct   (scaled G)
                g_sb = gpool.tile([P, P], mybir.dt.float32, name=f"g_sb_{b}", tag="g")
                nc.vector.scalar_tensor_tensor(
                    out=g_sb[:],
                    in0=g1_ps[:, 0:P],
                    scalar=INV_S,
                    in1=cs[:],
                    op0=mybir.AluOpType.mult,
                    op1=mybir.AluOpType.add,
                )
                g2_sb = gpool.tile([P, P], mybir.dt.float32, name=f"g2_{b}", tag="g2")
                nc.vector.tensor_add(g2_sb[:], g_sb[:], ct_ps[:])

                _shift_combine(nc, gpool, psh, j64_sb, g2_sb, o_all, b)

            nc.scalar.dma_start(out=out[:], in_=o_all[:])

    nc.compile()
    return nc


def _build_nc_1s(dt_in):
    """Single-stream Gram (fp16 or fp32 k-tiles), one MM per k-tile."""
    nc = bacc.Bacc("TRN2", target_bir_lowering=False, debug=False)

    xh = nc.dram_tensor("xh", [BPC * S * D2], dt_in, kind="ExternalInput")
    j64 = nc.dram_tensor("j64", [P, D], mybir.dt.float32, kind="ExternalInput")
    out = nc.dram_tensor("out", [D, BPC, 2, D], mybir.dt.float32, kind="ExternalOutput")

    with tile.TileContext(nc) as tc:
        with (
            tc.tile_pool(name="consts", bufs=1) as consts,
            tc.tile_pool(name="zpool", bufs=6) as zpool,
            tc.tile_pool(name="gpool", bufs=2) as gpool,
            tc.tile_pool(name="opool", bufs=1) as opool,
            tc.tile_pool(name="psg", bufs=2, space="PSUM") as psg,
            tc.tile_pool(name="psh", bufs=2, space="PSUM") as psh,
        ):
            j64_sb = consts.tile([P, D], mybir.dt.float32)
            nc.scalar.dma_start(out=j64_sb[:], in_=j64[:])
            o_all = opool.tile([D, BPC, 2, D], mybir.dt.float32)

            off = 0
            for b in range(BPC):
                zc = []
                for ci, nt in enumerate(CHUNKS_1[b]):
                    z = zpool.tile(
                        [P, nt, D2], dt_in, name=f"z_{b}_{ci}", tag="z"
                    )
                    n = nt * P * D2
                    nc.sync.dma_start(
                        out=z[:],
                        in_=xh[off : off + n].rearrange(
                            "(p t c) -> p t c", p=P, t=nt
                        ),
                    )
                    off += n
                    zc.append((z, nt))

                g_ps = psg.tile([P, P], mybir.dt.float32)
                first = True
                nchunks = len(zc)
                for ci, (z, nt) in enumerate(zc):
                    for t in range(nt):
                        zt = z[:, t, :]
                        nc.tensor.matmul(
                            g_ps[:], zt, zt,
                            start=first,
                            stop=(ci == nchunks - 1 and t == nt - 1),
                        )
                        first = False

                g_sb = gpool.tile([P, P], mybir.dt.float32, name=f"g_sb_{b}", tag="g")
                nc.vector.tensor_scalar_mul(g_sb[:], g_ps[:], INV_S)
                _shift_combine(nc, gpool, psh, j64_sb, g_sb, o_all, b)

            nc.scalar.dma_start(out=out[:], in_=o_all[:])

    nc.compile()
    return nc


def _flat_chunks(patterns):
    """Yield (b, ci, nt, off, first_of_batch, last_of_batch) over batches."""
    off = 0
    for b in range(BPC):
        n = len(patterns[b])
        for ci, nt in enumerate(patterns[b]):
            yield b, ci, nt, off, ci == 0, ci == n - 1
            off += nt * P * D2


def _build_nc_fp16_raw():
    """Hand-synchronized raw-bass fp16 Gram: no Tile boot/teardown cost.

    Sync engine: 9 chunk DMAs (unique SBUF slot each, FIFO ring).
    Tensor: per batch 64 accumulating MMs (+ J-shift MM, scheduled after
    the next batch's first chunk to hide the DVE round-trip).
    Vector: per batch scale-copy of G then the two block combines.
    Scalar: consts in, one packed output DMA out.
    """
    from contextlib import ExitStack

    nc = bacc.Bacc("TRN2", target_bir_lowering=False, debug=False)

    xh = nc.dram_tensor("xh", [BPC * S * D2], mybir.dt.float16, kind="ExternalInput")
    j64 = nc.dram_tensor("j64", [P, D], mybir.dt.float32, kind="ExternalInput")
    out = nc.dram_tensor("out", [D, BPC, 2, D], mybir.dt.float32, kind="ExternalOutput")

    chunks = list(_flat_chunks(CHUNKS_1))
    NCH = len(chunks)

    with ExitStack() as es:
        e = es.enter_context
        z = [
            e(nc.sbuf_tensor(f"z{k}", [P, nt, D2], mybir.dt.float16))
            for k, (_, _, nt, _, _, _) in enumerate(chunks)
        ]
        g_ps = [e(nc.psum_tensor(f"gps{i}", [P, P], mybir.dt.float32)) for i in range(2)]
        h_ps = [e(nc.psum_tensor(f"hps{i}", [D, P], mybir.dt.float32)) for i in range(2)]
        g_sb = [e(nc.sbuf_tensor(f"gsb{i}", [P, P], mybir.dt.float32)) for i in range(2)]
        o_all = e(nc.sbuf_tensor("o_all", [D, BPC, 2, D], mybir.dt.float32))
        j64_sb = e(nc.sbuf_tensor("j64sb", [P, D], mybir.dt.float32))

        dsem = [e(nc.semaphore(f"d{k}")) for k in range(NCH)]
        csem = e(nc.semaphore("csem"))
        pe_g = e(nc.semaphore("pe_g"))
        vec_g = e(nc.semaphore("vec_g"))
        pe_h = e(nc.semaphore("pe_h"))
        vec_o = e(nc.semaphore("vec_o"))
        osem = e(nc.semaphore("osem"))

        with nc.Block() as block:

            @block.sync
            def _(sync):
                for k, (_, _, nt, off, _, _) in enumerate(chunks):
                    n = nt * P * D2
                    sync.dma_start(
                        out=z[k][:],
                        in_=xh[off : off + n].rearrange(
                            "(p t c) -> p t c", p=P, t=nt
                        ),
                    ).then_inc(dsem[k], 16)

            @block.scalar
            def _(scalar):
                scalar.dma_start(out=j64_sb[:], in_=j64[:]).then_inc(csem, 16)
                scalar.wait_ge(vec_o, BPC)
                scalar.dma_start(out=out[:], in_=o_all[:]).then_inc(osem, 16)
                scalar.wait_ge(osem, 16)

            @block.tensor
            def _(tensor):
                def jmm(b):
                    # h = J64^T G_b ; h_ps[b%2] free once batch b-2 combined
                    tensor.wait_ge(vec_g, b + 1)
                    if b >= 1:
                        tensor.wait_ge(vec_o, b)
                    if b == 0:
                        tensor.wait_ge(csem, 16)
                    tensor.matmul(
                        h_ps[b % 2][:], j64_sb[:], g_sb[b % 2][:],
                        start=True, stop=True, skip_group_check=True,
                    ).then_inc(pe_h, 1)

                for k, (b, ci, nt, off, first_c, last_c) in enumerate(chunks):
                    if first_c and b >= 2:
                        tensor.wait_ge(vec_g, b - 1)  # g_ps[b%2] drained
                    tensor.wait_ge(dsem[k], 16)
                    for t in range(nt):
                        zt = z[k][:, t, :]
                        mm = tensor.matmul(
                            g_ps[b % 2][:], zt, zt,
                            start=(first_c and t == 0),
                            stop=(last_c and t == nt - 1),
                            skip_group_check=True,
                        )
                        if last_c and t == nt - 1:
                            mm.then_inc(pe_g, 1)
                    if first_c and b >= 1:
                        jmm(b - 1)  # hide DVE round-trip behind this chunk
                jmm(BPC - 1)

            @block.vector
            def _(vector):
                for b in range(BPC):
                    vector.wait_ge(pe_g, b + 1)
                    nc.vector.tensor_scalar_mul(
                        g_sb[b % 2][:], g_ps[b % 2][:], INV_S
                    ).then_inc(vec_g, 1)
                    vector.wait_ge(pe_h, b + 1)
                    nc.vector.tensor_add(
                        o_all[:, b, 0, :],
                        g_sb[b % 2][0:D, 0:D],
                        h_ps[b % 2][:, D : 2 * D],
                    )
                    nc.vector.tensor_sub(
                        o_all[:, b, 1, :],
                        g_sb[b % 2][0:D, D : 2 * D],
                        h_ps[b % 2][:, 0:D],
                    ).then_inc(vec_o, 1)

    nc.compile()
    return nc


def _build_nc_hl_raw():
    """Raw-bass fp16 hi/lo 2-matmul variant (fp32-grade accuracy)."""
    from contextlib import ExitStack

    nc = bacc.Bacc("TRN2", target_bir_lowering=False, debug=False)

    xh = nc.dram_tensor(
        "xh", [BPC * S * 2 * D2], mybir.dt.float16, kind="ExternalInput"
    )
    j64 = nc.dram_tensor("j64", [P, D], mybir.dt.float32, kind="ExternalInput")
    id128 = nc.dram_tensor("id128", [P, P], mybir.dt.float32, kind="ExternalInput")
    out = nc.dram_tensor("out", [D, BPC, 2, D], mybir.dt.float32, kind="ExternalOutput")

    chunks = list(_flat_chunks(CHUNKS_2))
    NCH = len(chunks)
    NSLOT = 8
    MAXT = max(nt for (_, _, nt, _, _, _) in chunks)

    with ExitStack() as es:
        e = es.enter_context
        z = [
            e(nc.sbuf_tensor(f"z{i}", [P, MAXT, 2, D2], mybir.dt.float16))
            for i in range(NSLOT)
        ]
        g1_ps = [e(nc.psum_tensor(f"g1ps{i}", [P, 2 * P], mybir.dt.float32)) for i in range(2)]
        ct_ps = [e(nc.psum_tensor(f"ctps{i}", [P, P], mybir.dt.float32)) for i in range(2)]
        h_ps = [e(nc.psum_tensor(f"hps{i}", [D, P], mybir.dt.float32)) for i in range(2)]
        cs_sb = [e(nc.sbuf_tensor(f"cssb{i}", [P, P], mybir.dt.float32)) for i in range(2)]
        g2_sb = [e(nc.sbuf_tensor(f"g2sb{i}", [P, P], mybir.dt.float32)) for i in range(2)]
        o_all = e(nc.sbuf_tensor("o_all", [D, BPC, 2, D], mybir.dt.float32))
        j64_sb = e(nc.sbuf_tensor("j64sb", [P, D], mybir.dt.float32))
        id_sb = e(nc.sbuf_tensor("idsb", [P, P], mybir.dt.float32))

        dsem = [e(nc.semaphore(f"d{k}")) for k in range(NCH)]
        cons = e(nc.semaphore("cons"))
        csem = e(nc.semaphore("csem"))
        vec_cs = e(nc.semaphore("vec_cs"))
        pe_ct = e(nc.semaphore("pe_ct"))
        vec_g2 = e(nc.semaphore("vec_g2"))
        vec_st = e(nc.semaphore("vec_st"))
        pe_h = e(nc.semaphore("pe_h"))
        vec_o = e(nc.semaphore("vec_o"))
        osem = e(nc.semaphore("osem"))

        with nc.Block() as block:

            @block.sync
            def _(sync):
                for k, (_, _, nt, off, _, _) in enumerate(chunks):
                    if k >= NSLOT:
                        sync.wait_ge(cons, k - NSLOT + 1)
                    n = nt * P * 2 * D2
                    sync.dma_start(
                        out=z[k % NSLOT][:, :nt, :, :],
                        in_=xh[2 * off : 2 * off + n].rearrange(
                            "(p t h c) -> p t h c", p=P, t=nt, h=2
                        ),
                    ).then_inc(dsem[k], 16)

            @block.scalar
            def _(scalar):
                scalar.dma_start(out=j64_sb[:], in_=j64[:]).then_inc(csem, 16)
                scalar.dma_start(out=id_sb[:], in_=id128[:]).then_inc(csem, 16)
                scalar.wait_ge(vec_o, BPC)
                scalar.dma_start(out=out[:], in_=o_all[:]).then_inc(osem, 16)
                scalar.wait_ge(osem, 16)

            @block.tensor
            def _(tensor):
                def ctmm(b):
                    # ct = cs^T (needs id128)
                    tensor.wait_ge(vec_cs, b + 1)
                    if b == 0:
                        tensor.wait_ge(csem, 32)
                    if b >= 2:
                        tensor.wait_ge(vec_g2, b - 1)  # ct_ps[b%2] drained
                    tensor.transpose(
                        ct_ps[b % 2][:], cs_sb[b % 2][:], id_sb[:]
                    ).then_inc(pe_ct, 1)

                def jmm(b):
                    tensor.wait_ge(vec_g2, b + 1)
                    if b >= 1:
                        tensor.wait_ge(vec_o, b)
                    tensor.matmul(
                        h_ps[b % 2][:], j64_sb[:], g2_sb[b % 2][:],
                        start=True, stop=True, skip_group_check=True,
                    ).then_inc(pe_h, 1)

                for k, (b, ci, nt, off, first_c, last_c) in enumerate(chunks):
                    if first_c and b >= 2:
                        tensor.wait_ge(vec_cs, b - 1)  # g1_ps[b%2] cs read
                        tensor.wait_ge(vec_g2, b - 1)  # g1_ps[b%2] A read
                    tensor.wait_ge(dsem[k], 16)
                    for t in range(nt):
                        mm = tensor.matmul(
                            g1_ps[b % 2][:],
                            z[k % NSLOT][:, t, 0, :],
                            z[k % NSLOT][:, t, :, :],
                            start=(first_c and t == 0),
                            stop=(last_c and t == nt - 1),
                            skip_group_check=True,
                        )
                        if t == nt - 1:
                            mm.then_inc(cons, 1)
                    # hide DVE round-trips behind subsequent chunks
                    if b >= 1 and ci == 0:
                        ctmm(b - 1)
                    if b >= 1 and ci == 1:
                        jmm(b - 1)
                ctmm(BPC - 1)
                jmm(BPC - 1)

            @block.vector
            def _(vector):
                cum = 0
                for b in range(BPC):
                    cum += len(CHUNKS_2[b])
                    vector.wait_ge(cons, cum)
                    nc.vector.tensor_scalar_mul(
                        cs_sb[b % 2][:], g1_ps[b % 2][:, P : 2 * P], INV_S / LSCALE
                    ).then_inc(vec_cs, 1)
                    vector.wait_ge(pe_ct, b + 1)
                    if b >= 2:
                        vector.wait_ge(pe_h, b - 1)  # g2_sb[b%2] consumed
                    nc.vector.scalar_tensor_tensor(
                        out=g2_sb[b % 2][:],
                        in0=g1_ps[b % 2][:, 0:P],
                        scalar=INV_S,
                        in1=cs_sb[b % 2][:],
                        op0=mybir.AluOpType.mult,
                        op1=mybir.AluOpType.add,
                    ).then_inc(vec_st, 1)
                    vector.wait_ge(vec_st, b + 1)
                    nc.vector.tensor_add(
                        g2_sb[b % 2][:], g2_sb[b % 2][:], ct_ps[b % 2][:]
                    ).then_inc(vec_g2, 1)
                    vector.wait_ge(pe_h, b + 1)
                    nc.vector.tensor_add(
                        o_all[:, b, 0, :],
                        g2_sb[b % 2][0:D, 0:D],
                        h_ps[b % 2][:, D : 2 * D],
                    )
                    nc.vector.tensor_sub(
                        o_all[:, b, 1, :],
                        g2_sb[b % 2][0:D, D : 2 * D],
                        h_ps[b % 2][:, 0:D],
                    ).then_inc(vec_o, 1)

    nc.compile()
    return nc


def _j64_host():
    j = np.zeros((P, D), np.float32)
    j[D + np.arange(D), np.arange(D)] = 1.0
    return j


def _chunkify(a, patterns):
    """a: [BPC, S, ...tail] -> flat 1-D array in chunk layout.

    Chunk of nt k-tiles covering rows [base, base+nt*P): stored as
    [p, t, ...tail] with row = base + p*nt + t.
    """
    segs = []
    for b in range(BPC):
        base = 0
        for nt in patterns[b]:
            rows = nt * P
            seg = a[b, base : base + rows]          # [rows, ...tail]
            seg = seg.reshape(P, nt, *a.shape[2:])  # p-major
            segs.append(seg.reshape(-1))
            base += rows
    return np.concatenate(segs)


def _prep(xz):
    """Returns dict of per-core host arrays for the active VARIANT."""
    xzc = xz.reshape(N_CORES, BPC, S, D2)
    maps = []
    for c in range(N_CORES):
        a = xzc[c]
        if VARIANT in ("fp16", "fp16_raw"):
            m = {"xh": _chunkify(a.astype(np.float16), CHUNKS_1)}
        elif VARIANT == "fp32":
            m = {"xh": _chunkify(a, CHUNKS_1)}
        elif VARIANT == "fp16f8":
            zh = a.astype(np.float16)
            zl = ((a - zh.astype(np.float32)) * LSCALE).astype(
                ml_dtypes.float8_e4m3
            )
            m = {
                "xh": _chunkify(zh, CHUNKS_2),
                "xl": _chunkify(zl, CHUNKS_2),
            }
        elif VARIANT in ("fp16hl", "fp16hl_raw"):
            zh = a.astype(np.float16)
            zl = ((a - zh.astype(np.float32)) * LSCALE).astype(np.float16)
            zs = np.stack([zh, zl], axis=2)  # [BPC, S, 2, D2]
            m = {"xh": _chunkify(zs, CHUNKS_2)}
        else:
            raise ValueError(VARIANT)
        maps.append(m)
    return maps


def _build():
    if VARIANT == "fp16":
        return _build_nc_1s(mybir.dt.float16)
    if VARIANT == "fp16_raw":
        return _build_nc_fp16_raw()
    if VARIANT == "fp16hl_raw":
        return _build_nc_hl_raw()
    if VARIANT == "fp32":
        return _build_nc_1s(mybir.dt.float32)
    if VARIANT == "fp16f8":
        return _build_nc_hl(lo_fp8=True)
    if VARIANT == "fp16hl":
        return _build_nc_hl(lo_fp8=False)
    raise ValueError(VARIANT)


def kernel(input_real, input_imag):
    global LAST_RESULTS
    xr = np.asarray(input_real, dtype=np.float32)
    xi = np.asarray(input_imag, dtype=np.float32)
    assert xr.shape == (B, S, D) and xi.shape == (B, S, D)

    xz = np.concatenate([xr, xi], axis=2)  # [B, S, 2D]

    key = ("nc", VARIANT)
    if key not in _NC_CACHE:
        _NC_CACHE[key] = _build()
    nc = _NC_CACHE[key]

    maps = _prep(xz)
    j64 = _j64_host()
    ident = np.eye(P, dtype=np.float32)
    in_maps = []
    for c in range(N_CORES):
        m = dict(maps[c])
        m["j64"] = j64
        if VARIANT in ("fp16f8", "fp16hl", "fp16hl_raw"):
            m["id128"] = ident
        in_maps.append(m)
    tmpdir = os.environ.get("BASS_TMPDIR") or None
    res = run_bass_kernel_spmd(
        nc, in_maps, core_ids=list(range(N_CORES)), tmpdir=tmpdir
    )
    LAST_RESULTS = res

    # per-core out: [D, BPC, 2, D] -> [BPC, 2, D, D]
    outs = np.stack(
        [res.results[c]["out"].transpose(1, 2, 0, 3) for c in range(N_CORES)]
    )
    out = outs.reshape(B, 2, D, D)
    return np.ascontiguousarray(out[:, 0]), np.ascontiguousarray(out[:, 1])
